# revision 30
# baseline (speedup 1.0000x reference)
"""BiLSTM-CRF NLL loss on 8 Trainium2 NeuronCores.

Sharding: core c owns sequences [4c, 4c+4); each core runs BOTH LSTM
directions and the full CRF for its 4 sequences. No collectives; host sums
8 per-core partials.

Recurrence: TWO phase-shifted chains per core, chain ch owns seqs
{2ch, 2ch+1} and carries BOTH directions in merged instructions. Per
chain-step: one PSUM tile G [128, 64] (cols = d*32 + m*2 + s'), one
accumulation group: bias-inject identity mm (start=True) + 32 x-mms +
64 h-mms (stop on last). sigma-trick: g-gate rows of W/b pre-scaled by 2 so
ONE Sigmoid activation covers all 16 gate chunks (tanh(g) = 2*sigma(2g)-1);
cell update is 4 fused DVE ops (c = t1 + 2*i*s - i), then tanh(c) Act and
the h-mul DVE writes both directions' h slots via a strided AP. This cuts
the per-step serial chain to PE -> Act -> DVE -> Act -> DVE with two
independent chains pipelined to hide the cross-engine latency.

Weights fp8e4m3, x/h matmuls fp8 DoubleRow. h stored fp8 unit-major.

CRF partition function: exp-domain, two-sided (alpha ascends, beta descends,
meet at K=127), bf16 chain operands. Gold-path score via one-hot matmuls.
Loss exits as [1,4] per-core partials.

Self-contained: hardcodes all shapes; only needs numpy + concourse (+ml_dtypes).
"""
import numpy as np
import ml_dtypes

import concourse.bass as bass
import concourse.bacc as bacc
import concourse.tile as tile
from concourse import mybir
from concourse.tile_rust import add_dep_helper
from concourse.bass_utils import run_bass_kernel_spmd

F32 = mybir.dt.float32
FP8 = mybir.dt.float8e4
BF16 = mybir.dt.bfloat16
I32 = mybir.dt.int32
AF = mybir.ActivationFunctionType
ALU = mybir.AluOpType

B, S, E, H, T, V = 32, 256, 256, 512, 45, 50000
NS = 4                 # seqs per core
N = S * NS             # 1024 emission cols, n = 4t+s
NCH = 16               # gate chunks (2048/128)
HC = 4                 # h chunks (512/128)
SW = HC * NS           # state cols per step = 16 (all 4 seqs)
NB_T = 4               # transform n-blocks (of 64 steps = 256 cols each)
TBLK = S // NB_T       # 64 steps per transform block
LN45 = float(np.log(45.0))
DSTRIDE = (S + 1) * SW  # 4112: cols per direction in hsT_all

_cached = {}


def _build(stop_after=None):
    lv = {"xf": 1, "rec": 2, "em": 3, "crf": 4, None: 5}[stop_after]
    nc = bacc.Bacc("TRN2", target_bir_lowering=False, debug=False, num_devices=8)

    d = {}
    d["emb"] = nc.dram_tensor("emb", [V, E], BF16, kind="ExternalInput")
    d["xidx"] = nc.dram_tensor("xidx", [128, 8], I32, kind="ExternalInput")
    d["wihf"] = nc.dram_tensor("wihf", [128, 32 * 128], FP8, kind="ExternalInput")
    d["wihb"] = nc.dram_tensor("wihb", [128, 32 * 128], FP8, kind="ExternalInput")
    d["whhf"] = nc.dram_tensor("whhf", [128, 64 * 128], FP8, kind="ExternalInput")
    d["whhb"] = nc.dram_tensor("whhb", [128, 64 * 128], FP8, kind="ExternalInput")
    d["biasbc"] = nc.dram_tensor("biasbc", [128, 128], BF16, kind="ExternalInput")
    d["linT"] = nc.dram_tensor("linT", [128, 8 * T], BF16, kind="ExternalInput")
    d["linb"] = nc.dram_tensor("linb", [T, 1], F32, kind="ExternalInput")
    d["id128"] = nc.dram_tensor("id128", [128, 128], F32, kind="ExternalInput")
    d["idbf"] = nc.dram_tensor("idbf", [128, 128], BF16, kind="ExternalInput")
    d["trans"] = nc.dram_tensor("trans", [T, T], F32, kind="ExternalInput")
    d["stend"] = nc.dram_tensor("stend", [T, 2], F32, kind="ExternalInput")
    d["epstk"] = nc.dram_tensor("epstk", [128, 128], BF16, kind="ExternalInput")
    d["epstkT"] = nc.dram_tensor("epstkT", [128, 128], BF16, kind="ExternalInput")
    d["i45stk"] = nc.dram_tensor("i45stk", [128, 45], BF16, kind="ExternalInput")
    d["stendstk"] = nc.dram_tensor("stendstk", [128, 2], F32, kind="ExternalInput")
    d["linbstk"] = nc.dram_tensor("linbstk", [128, 1], F32, kind="ExternalInput")
    d["oh"] = nc.dram_tensor("oh", [T, N], F32, kind="ExternalInput")
    d["oh2"] = nc.dram_tensor("oh2", [T, N], F32, kind="ExternalInput")
    d_loss = nc.dram_tensor("loss", [1, NS], F32, kind="ExternalOutput")

    with tile.TileContext(nc) as tc:
        with tc.tile_pool(name="persist", bufs=1) as pp, \
             tc.tile_pool(name="gxp", bufs=1) as gxp:
            # persistent weights / tables
            wih = {0: pp.tile([128, 32 * 128], FP8, tag="wihf", name="wihf"),
                   1: pp.tile([128, 32 * 128], FP8, tag="wihb", name="wihb")}
            whh = {0: pp.tile([128, 64 * 128], FP8, tag="whhf", name="whhf"),
                   1: pp.tile([128, 64 * 128], FP8, tag="whhb", name="whhb")}
            biasbc = pp.tile([128, 128], BF16, tag="biasbc")
            ones1 = pp.tile([1, NS], F32, tag="ones1")
            id128 = pp.tile([128, 128], F32, tag="id128")
            idbf = pp.tile([128, 128], BF16, tag="idbf")
            xidx = pp.tile([128, 8], I32, tag="xidx")
            linT = pp.tile([128, 8 * T], BF16, tag="linT")
            linb = pp.tile([T, 1], F32, tag="linb")
            epstk = pp.tile([128, 128], BF16, tag="epstk")
            epstkT = pp.tile([128, 128], BF16, tag="epstkT")
            i45stk = pp.tile([128, 45], BF16, tag="i45stk")
            stendstk = pp.tile([128, 2], F32, tag="stendstk")
            # estk[pair]: exp(emissions), seq-pair stacked on partitions
            # (second seq at partition 64), one column per position
            estk = {0: pp.tile([128, S], F32, tag="estk0", name="estk0"),
                    1: pp.tile([128, S], F32, tag="estk1", name="estk1")}
            em_lin = pp.tile([T, N], F32, tag="em_lin")
            nc.sync.dma_start(out=xidx[:], in_=d["xidx"][:])
            nc.sync.dma_start(out=id128[:], in_=d["id128"][:])
            nc.sync.dma_start(out=idbf[:], in_=d["idbf"][:])
            nc.sync.dma_start(out=biasbc[:], in_=d["biasbc"][:])
            nc.sync.dma_start(out=linb[:], in_=d["linb"][:])
            nc.sync.dma_start(out=epstk[:], in_=d["epstk"][:])
            nc.sync.dma_start(out=epstkT[:], in_=d["epstkT"][:])
            nc.sync.dma_start(out=i45stk[:], in_=d["i45stk"][:])
            nc.sync.dma_start(out=stendstk[:], in_=d["stendstk"][:])
            nc.vector.memset(ones1[:], 1.0)

            # XT block tiles: [nb] -> [128, 2 ec x 256 n] fp8
            xt = {nb: gxp.tile([128, 2 * TBLK * NS], FP8, tag=f"xt{nb}", name=f"xt{nb}")
                  for nb in range(NB_T)}
            # h state, both dirs in one tile (unit-major):
            # col = d*DSTRIDE + 16*slot + 4*k + s ; slot S = zeros (h0).
            # BOTH directions write slot u at recurrence step u: the bwd
            # direction's h (position S-1-u) is stored TIME-REVERSED, so
            # h-mm reads and the h-write are uniform across dirs. The
            # reversal is undone in the emissions phase via a reversed AP.
            hsT = pp.tile([128, 2 * DSTRIDE], FP8, tag="hsT", name="hsT")
            nc.vector.memset(hsT[:, SW * S: SW * S + SW], 0.0)
            nc.vector.memset(hsT[:, DSTRIDE + SW * S: DSTRIDE + SW * S + SW], 0.0)

            # ---------- phase 0: gather + transpose -> XT ----------
            with tc.tile_pool(name="gat", bufs=3) as gp, \
                 tc.tile_pool(name="ps_tp", bufs=4, space="PSUM") as ps_tp:
                for b in range(8):
                    X = gp.tile([128, E], BF16, tag="X")
                    nc.gpsimd.indirect_dma_start(
                        out=X[:],
                        out_offset=None,
                        in_=d["emb"][:],
                        in_offset=bass.IndirectOffsetOnAxis(ap=xidx[:, b:b + 1], axis=0),
                    )
                    nb, off = b // 2, (b % 2) * 128
                    for ec in range(2):
                        tp = ps_tp.tile([128, 128], BF16, tag="tp")
                        nc.tensor.transpose(tp[:], X[:, 128 * ec: 128 * ec + 128], idbf[:])
                        nc.vector.tensor_copy(
                            xt[nb][:, TBLK * NS * ec + off: TBLK * NS * ec + off + 128],
                            tp[:])

            # weight DMAs after the gathers so they share the DMA engines
            nc.sync.dma_start(out=wih[0][:], in_=d["wihf"][:])
            nc.sync.dma_start(out=wih[1][:], in_=d["wihb"][:])
            nc.sync.dma_start(out=whh[0][:], in_=d["whhf"][:])
            nc.sync.dma_start(out=whh[1][:], in_=d["whhb"][:])
            nc.sync.dma_start(out=linT[:], in_=d["linT"][:])

            # ---------- recurrence ----------
            if lv == 1:
                probe = pp.tile([1, NS], F32, tag="probe")
                nc.vector.tensor_copy(probe[:], xt[0][0:1, 0:NS])
                nc.sync.dma_start(out=d_loss[:], in_=probe[:])
            zres = pp.tile([1, NS], F32, tag="zres")      # sum_j u1*v1 per seq
            em_stk = {0: pp.tile([128, S], F32, tag="emstk0", name="emstk0"),
                      1: pp.tile([128, S], F32, tag="emstk1", name="emstk1")}
            linbstk = pp.tile([128, 1], F32, tag="linbstk")
            nc.sync.dma_start(out=linbstk[:], in_=d["linbstk"][:])
            if lv >= 2:
                cfp = tc.alloc_tile_pool(name="crfp", bufs=3)
                cur = {}
                with tc.tile_pool(name="rec0", bufs=6) as rp0, \
                     tc.tile_pool(name="rec1", bufs=6) as rp1, \
                     tc.tile_pool(name="psg0", bufs=2, space="PSUM") as pg0, \
                     tc.tile_pool(name="psg1", bufs=2, space="PSUM") as pg1, \
                     tc.tile_pool(name="psem", bufs=1, space="PSUM") as ps_em, \
                     tc.tile_pool(name="pscrf", bufs=1, space="PSUM") as ps_crf:
                    rp = [rp0, rp1]
                    pg = [pg0, pg1]
                    cprev = [None, None]
                    for ch in (0, 1):
                        cinit = rp[ch].tile([128, 16], BF16, tag="c")
                        nc.vector.memset(cinit[:], 0.0)
                        cprev[ch] = cinit

                    DR = mybir.MatmulPerfMode.DoubleRow
                    xtv = {nb: xt[nb].rearrange("p (e c) -> p e c", e=2)
                           for nb in range(NB_T)}
                    hv = hsT.rearrange("p (a t k s) -> p a t k s",
                                       a=2, t=S + 1, k=HC, s=NS)
                    def stage_pe(dd, u):
                        # chain dd = ONE direction, all 4 seqs. Both dirs
                        # write slot u, read slot u-1 (u=0: slot S = zeros);
                        # bwd h is stored time-reversed.
                        slot_r = S if u == 0 else u - 1
                        col = u if dd == 0 else S - 1 - u   # xt timestep

                        G = pg[dd].tile([128, 64], F32, tag="G")
                        # one accumulation group per G tile: bias-inject
                        # (start=True zeroes the region) -> x-mms -> h-mms
                        # (stop=True on the very last; at u=0 there are no
                        # h-mms so the last x-mm stops). add_dep_helper pins
                        # start-first / stop-last against scheduler
                        # reordering. Phase A (inject + x) has no h dep and
                        # runs in the previous step's tail.
                        mms = []
                        mm = nc.tensor.matmul(
                            G[:], idbf[:],
                            biasbc[:, dd * 64: dd * 64 + 64],
                            start=True, stop=False)
                        mms.append(mm)
                        nb = col // TBLK
                        j = col % TBLK
                        for m in range(NCH):
                            sl = G[:, 4 * m: 4 * m + 4]
                            wpair = wih[dd][:, 2 * m * 128:(2 * m + 2) * 128]
                            mm = nc.tensor.matmul(
                                sl,
                                wpair.rearrange("p (c f) -> p c f", c=2),
                                xtv[nb][:, :, NS * j: NS * j + NS],
                                start=False,
                                stop=(u == 0 and m == NCH - 1),
                                perf_mode=DR)
                            mms.append(mm)
                        if u > 0:
                            for m in range(NCH):
                                sl = G[:, 4 * m: 4 * m + 4]
                                for kp in range(HC // 2):
                                    hpair = whh[dd][
                                        :, (4 * m + 2 * kp) * 128:
                                        (4 * m + 2 * kp + 2) * 128]
                                    rhs = hv[:, dd, slot_r,
                                             2 * kp: 2 * kp + 2, :]
                                    mm = nc.tensor.matmul(
                                        sl,
                                        hpair.rearrange(
                                            "p (c f) -> p c f", c=2),
                                        rhs,
                                        start=False,
                                        stop=(m == NCH - 1
                                              and kp == HC // 2 - 1),
                                        perf_mode=DR)
                                    mms.append(mm)
                        first, last = mms[0], mms[-1]
                        for mm in mms[1:]:
                            add_dep_helper(mm.ins, first.ins, sync=False,
                                           reason="group start first")
                        for mm in mms[:-1]:
                            add_dep_helper(last.ins, mm.ins, sync=False,
                                           reason="group stop last")
                        return G

                    def stage_sig(dd, G):
                        # sigma over ALL 64 gate cols (g rows pre-scaled x2)
                        SG = rp[dd].tile([128, 64], BF16, tag="SG")
                        nc.scalar.activation(SG[:], G[:], AF.Sigmoid)
                        return SG

                    def stage_cell(dd, SG):
                        SGv = SG.rearrange("p (m s) -> p m s", m=NCH)
                        ihat = SGv[:, 0:4, :]
                        fhat = SGv[:, 4:8, :]
                        shat = SGv[:, 12:16, :]
                        t1 = rp[dd].tile([128, 16], BF16, tag="t1")
                        t2 = rp[dd].tile([128, 16], BF16, tag="t2")
                        cnext = rp[dd].tile([128, 16], BF16, tag="c")
                        cpv = cprev[dd].rearrange("p (k s) -> p k s", k=4)
                        t1v = t1.rearrange("p (k s) -> p k s", k=4)
                        t2v = t2.rearrange("p (k s) -> p k s", k=4)
                        cnv = cnext.rearrange("p (k s) -> p k s", k=4)
                        # c = fhat*c_prev + ihat*tanh(g)
                        #   = t1 + 2*((shat - 0.5)*ihat)   [tanh(g)=2s-1]
                        nc.vector.tensor_mul(t1v[:], fhat, cpv[:])
                        nc.vector.scalar_tensor_tensor(
                            out=t2v[:], in0=shat, scalar=0.5, in1=ihat,
                            op0=ALU.subtract, op1=ALU.mult)
                        nc.vector.scalar_tensor_tensor(
                            out=cnv[:], in0=t2v[:], scalar=2.0, in1=t1v[:],
                            op0=ALU.mult, op1=ALU.add)
                        cprev[dd] = cnext
                        return cnext

                    def stage_tanh(dd, cnext):
                        th = rp[dd].tile([128, 16], BF16, tag="th")
                        nc.scalar.activation(th[:], cnext[:], AF.Tanh)
                        return th

                    def stage_h(dd, u, SG, th):
                        SGv = SG.rearrange("p (m s) -> p m s", m=NCH)
                        ohat = SGv[:, 8:12, :]
                        hout = hv[:, dd, u, :, :]
                        thv = th.rearrange("p (k s) -> p k s", k=4)
                        nc.vector.tensor_mul(hout, ohat, thv[:])

                    # ---- incremental emissions + mid-out CRF machinery ----
                    # At step u>=128, positions u and S-1-u complete. Every
                    # 8 steps two 8-position blocks (A: ascending from 128,
                    # B: descending from 127) get emissions computed into
                    # the seq-pair-stacked em_stk/estk layout. The CRF
                    # partition function is two matrix-product chains per
                    # seq pair (R consumes t=128..255 ascending, L~ consumes
                    # t=127..1 descending), one link per chain per step,
                    # hidden behind the recurrence.
                    def emit_block(p0, fslot0, bslot0):
                        # emissions for positions [p0, p0+8); bwd slots
                        # [bslot0, bslot0+8) ascending = positions reversed.
                        # One PSUM tile per pair: cols 0-7 fwd, 8-15 bwd,
                        # one accumulation group.
                        for pr in (0, 1):
                            pe = ps_em.tile([128, 16], F32, tag=f"pe{pr}")
                            emms = []
                            for c0_, dd, sl0, kb in ((0, 0, fslot0, 0),
                                                     (8, 1, bslot0, 4)):
                                for sh in (0, 1):
                                    seq = 2 * pr + sh
                                    out = pe[64 * sh: 64 * sh + T,
                                             c0_: c0_ + 8]
                                    for k in range(4):
                                        kc = kb + k
                                        emms.append(nc.tensor.matmul(
                                            out,
                                            linT[:, T * kc: T * (kc + 1)],
                                            hv[:, dd, sl0:sl0 + 8, k, seq],
                                            start=(c0_ == 0 and sh == 0
                                                   and k == 0),
                                            stop=(c0_ == 8 and sh == 1
                                                  and k == 3)))
                            first, last = emms[0], emms[-1]
                            for m_ in emms[1:]:
                                add_dep_helper(m_.ins, first.ins, sync=False,
                                               reason="em start first")
                            for m_ in emms[:-1]:
                                add_dep_helper(last.ins, m_.ins, sync=False,
                                               reason="em stop last")
                            pb_s = cfp.tile([128, 8], F32, tag=f"pbs{pr}")
                            nc.vector.tensor_copy(pb_s[:], pe[:, 8:16])
                            nc.vector.tensor_tensor(
                                out=em_stk[pr][:, p0:p0 + 8],
                                in0=pe[:, 0:8],
                                in1=pb_s[:, ::-1], op=ALU.add)
                            nc.scalar.activation(
                                estk[pr][:, p0:p0 + 8],
                                em_stk[pr][:, p0:p0 + 8], AF.Exp,
                                bias=linbstk[:])

                    cur.update({("L", 0): i45stk, ("L", 1): i45stk,
                                ("R", 0): i45stk, ("R", 1): i45stk})

                    def link(kind, pr, t):
                        # one CRF chain link: N_t = diag(e_t) Ep^T
                        ecol = estk[pr][:, t:t + 1]
                        ps = ps_crf.tile([128, T], F32, tag=f"ps{pr}")
                        if kind == "R":
                            # R_k = Ep^T R_{k-1}, then row-scale by e_t
                            nc.tensor.matmul(ps[:], epstk[:], cur[(kind, pr)][:],
                                             start=True, stop=True)
                            nxt = cfp.tile([128, T], BF16, tag=f"cR{pr}")
                            nc.vector.tensor_scalar_mul(nxt[:], ps[:], ecol)
                        else:
                            # L~_k = Ep (diag(e_t) L~_{k-1})
                            rs = cfp.tile([128, T], BF16, tag=f"rL{pr}")
                            nc.vector.tensor_scalar_mul(
                                rs[:], cur[(kind, pr)][:], ecol)
                            nc.tensor.matmul(ps[:], epstkT[:], rs[:],
                                             start=True, stop=True)
                            nxt = cfp.tile([128, T], BF16, tag=f"cL{pr}")
                            nc.vector.tensor_copy(nxt[:], ps[:])
                        cur[(kind, pr)] = nxt

                    # stage-major emission: each engine's stream is ordered
                    # by expected data-arrival time so the in-order engine
                    # FIFOs never head-block across the two chains.
                    r_next, r_avail = [128], [127]
                    l_next, l_avail = [127], [128]

                    for u in range(S):
                        G0 = stage_pe(0, u)
                        G1 = stage_pe(1, u)
                        SG0 = stage_sig(0, G0)
                        SG1 = stage_sig(1, G1)
                        c0 = stage_cell(0, SG0)
                        c1 = stage_cell(1, SG1)
                        th0 = stage_tanh(0, c0)
                        th1 = stage_tanh(1, c1)
                        stage_h(0, u, SG0, th0)
                        stage_h(1, u, SG1, th1)
                        if lv >= 3:
                            if u >= 135 and u % 8 == 7:
                                b = (u - 135) // 8
                                emit_block(128 + 8 * b, 128 + 8 * b,
                                           120 - 8 * b)
                                emit_block(120 - 8 * b, 120 - 8 * b,
                                           128 + 8 * b)
                                r_avail[0] = 135 + 8 * b
                                l_avail[0] = 120 - 8 * b
                            if r_next[0] <= r_avail[0]:
                                link("R", 0, r_next[0])
                                link("R", 1, r_next[0])
                                r_next[0] += 1
                            if l_next[0] >= max(l_avail[0], 1):
                                link("L", 0, l_next[0])
                                link("L", 1, l_next[0])
                                l_next[0] -= 1

                    if lv >= 3:
                        # drain remaining links after the recurrence
                        while r_next[0] <= 255:
                            link("R", 0, r_next[0])
                            link("R", 1, r_next[0])
                            r_next[0] += 1
                        while l_next[0] >= 1:
                            link("L", 0, l_next[0])
                            link("L", 1, l_next[0])
                            l_next[0] -= 1

                if lv >= 3:
                    with tc.tile_pool(name="psfin", bufs=1,
                                      space="PSUM") as ps_fin:
                        # Z = eend^T R L alpha0, alpha0 = exp(start) * e_0
                        esstk = cfp.tile([128, 2], BF16, tag="esstk")
                        nc.scalar.activation(esstk[:], stendstk[:], AF.Exp)
                        v1p = ps_fin.tile([T, NS], F32, tag="v1p")
                        u1p = ps_fin.tile([T, NS], F32, tag="u1p")
                        fmms_v, fmms_u = [], []
                        a0 = {}
                        for pr in (0, 1):
                            a0[pr] = cfp.tile([128, 1], BF16, tag=f"a0{pr}",
                                              name=f"a0{pr}")
                            nc.vector.tensor_mul(a0[pr][:], esstk[:, 0:1],
                                                 estk[pr][:, 0:1])
                        for s in range(NS):
                            pr, sh = s // 2, s % 2
                            r0 = 64 * sh
                            fmms_v.append(nc.tensor.matmul(
                                v1p[:, s:s + 1],
                                cur[("L", pr)][r0:r0 + T, :],
                                a0[pr][r0:r0 + T, :],
                                start=(s == 0), stop=(s == NS - 1)))
                            fmms_u.append(nc.tensor.matmul(
                                u1p[:, s:s + 1],
                                cur[("R", pr)][r0:r0 + T, :],
                                esstk[r0:r0 + T, 1:2],
                                start=(s == 0), stop=(s == NS - 1)))
                        for fm in (fmms_v, fmms_u):
                            for m_ in fm[1:]:
                                add_dep_helper(m_.ins, fm[0].ins, sync=False,
                                               reason="z start first")
                            for m_ in fm[:-1]:
                                add_dep_helper(fm[-1].ins, m_.ins, sync=False,
                                               reason="z stop last")
                        v1s = cfp.tile([T, NS], F32, tag="v1s")
                        wz = cfp.tile([T, NS], F32, tag="wz")
                        nc.vector.tensor_copy(v1s[:], v1p[:])
                        nc.vector.tensor_mul(wz[:], v1s[:], u1p[:])
                        zp = ps_fin.tile([1, NS], F32, tag="zp")
                        ones45r = cfp.tile([T, 1], F32, tag="ones45r")
                        nc.vector.memset(ones45r[:], 1.0)
                        nc.tensor.matmul(zp[:], ones45r[:], wz[:],
                                         start=True, stop=True)
                        nc.vector.tensor_copy(zres[:], zp[:])

                        # de-stack em into em_lin [T, 4t+s] for gold score
                        for pr in (0, 1):
                            for sh in (0, 1):
                                seq = 2 * pr + sh
                                emv = em_lin.rearrange("p (t s) -> p t s",
                                                       s=NS)
                                nc.sync.dma_start(
                                    out=emv[:, :, seq],
                                    in_=em_stk[pr][64 * sh: 64 * sh + T, :])

                if lv == 2:
                    probe = pp.tile([1, NS], F32, tag="probe")
                    nc.vector.tensor_copy(probe[:], hsT[0:1, 0:NS])
                    nc.sync.dma_start(out=d_loss[:], in_=probe[:])

            if lv >= 2:
                cfp.release()

            # ---------- gold score + final loss ----------
            if lv == 3:
                probe = pp.tile([1, NS], F32, tag="probe")
                nc.vector.tensor_copy(probe[:], em_lin[0:1, 0:NS])
                nc.sync.dma_start(out=d_loss[:], in_=probe[:])
            if lv >= 4:
                with tc.tile_pool(name="crf", bufs=1) as cp, \
                     tc.tile_pool(name="ps_f", bufs=1, space="PSUM") as ps_f:
                    trans_sb = cp.tile([T, T], F32, tag="trans")
                    stend = cp.tile([T, 2], F32, tag="stend")
                    ones45 = cp.tile([T, 1], F32, tag="ones45")
                    oh = cp.tile([T, N], F32, tag="oh")
                    oh2 = cp.tile([T, N], F32, tag="oh2")
                    nc.sync.dma_start(out=trans_sb[:], in_=d["trans"][:])
                    nc.sync.dma_start(out=stend[:], in_=d["stend"][:])
                    nc.sync.dma_start(out=oh[:], in_=d["oh"][:])
                    nc.sync.dma_start(out=oh2[:], in_=d["oh2"][:])
                    nc.vector.memset(ones45[:], 1.0)

                    logZ = cp.tile([1, NS], F32, tag="logZ")
                    em_h = cp.tile([1, 2 * NS], F32, tag="em_h")
                    tr_h = cp.tile([1, 2 * NS], F32, tag="tr_h")
                    em_sc = cp.tile([1, NS], F32, tag="em_sc")
                    tr_sc = cp.tile([1, NS], F32, tag="tr_sc")
                    sten_s = cp.tile([1, NS], F32, tag="sten_s")
                    nc.scalar.activation(logZ[:], zres[:], AF.Ln)

                    # S1 = (em_lin + linb) * onehot(tags)
                    S1 = cp.tile([T, N], F32, tag="S1")
                    nc.vector.scalar_tensor_tensor(
                        out=S1[:], in0=em_lin[:], scalar=linb[:], in1=oh[:],
                        op0=ALU.add, op1=ALU.mult)
                    S2 = cp.tile([T, N], F32, tag="S2")
                    for ck in range(2):
                        sl = slice(512 * ck, 512 * (ck + 1))
                        s1p = ps_f.tile([1, 512], F32, tag="fbig")
                        nc.tensor.matmul(s1p[:], ones45[:], S1[:, sl],
                                         start=True, stop=True)
                        nc.vector.tensor_reduce(
                            em_h[:, NS * ck: NS * (ck + 1)],
                            s1p.rearrange("p (t b) -> p b t", b=NS),
                            axis=mybir.AxisListType.X, op=ALU.add)
                        Rp_ = ps_f.tile([T, 512], F32, tag="fR")
                        nc.tensor.matmul(Rp_[:], trans_sb[:], oh[:, sl],
                                         start=True, stop=True)
                        nc.vector.tensor_mul(S2[:, sl], Rp_[:], oh2[:, sl])
                        s2p = ps_f.tile([1, 512], F32, tag="fbig2")
                        nc.tensor.matmul(s2p[:], ones45[:], S2[:, sl],
                                         start=True, stop=True)
                        nc.vector.tensor_reduce(
                            tr_h[:, NS * ck: NS * (ck + 1)],
                            s2p.rearrange("p (t b) -> p b t", b=NS),
                            axis=mybir.AxisListType.X, op=ALU.add)
                    nc.vector.tensor_add(em_sc[:], em_h[:, 0:NS],
                                         em_h[:, NS:2 * NS])
                    nc.vector.tensor_add(tr_sc[:], tr_h[:, 0:NS],
                                         tr_h[:, NS:2 * NS])

                    stp = cp.tile([T, NS], F32, tag="stp")
                    enp = cp.tile([T, NS], F32, tag="enp")
                    nc.vector.tensor_scalar_mul(stp[:], oh[:, 0:NS],
                                                stend[:, 0:1])
                    nc.vector.tensor_scalar_mul(enp[:], oh[:, N - NS:N],
                                                stend[:, 1:2])
                    sten = ps_f.tile([1, NS], F32, tag="f2")
                    nc.tensor.matmul(sten[:], ones45[:], stp[:],
                                     start=True, stop=False)
                    nc.tensor.matmul(sten[:], ones45[:], enp[:],
                                     start=False, stop=True)
                    nc.vector.tensor_copy(sten_s[:], sten[:])

                    sc1 = cp.tile([1, NS], F32, tag="sc1")
                    sc2 = cp.tile([1, NS], F32, tag="sc2")
                    lossa = cp.tile([1, NS], F32, tag="lossa")
                    lossb = cp.tile([1, NS], F32, tag="lossb")
                    nc.vector.tensor_add(sc1[:], em_sc[:], tr_sc[:])
                    nc.vector.tensor_add(sc2[:], sc1[:], sten_s[:])
                    nc.vector.tensor_tensor(out=lossa[:], in0=logZ[:],
                                            in1=sc2[:], op=ALU.subtract)
                    nc.scalar.activation(lossb[:], lossa[:], AF.Copy,
                                         bias=(S - 1) * LN45)
                    nc.sync.dma_start(out=d_loss[:], in_=lossb[:])

    nc.finalize()
    return nc


def _pack_wT(w, kchunks):
    # w: [M_out rows (gate units, reordered), K] ->
    # [128, (nm*kchunks)*128] tiles: tile (m*kchunks+ec) = w[mU, ecK].T
    M, K = w.shape
    nm = M // 128
    assert K == 128 * kchunks
    tiles = []
    for m in range(nm):
        for ec in range(kchunks):
            blk = w[m * 128:(m + 1) * 128, ec * 128:(ec + 1) * 128]
            tiles.append(np.ascontiguousarray(blk.T))
    return np.concatenate(tiles, axis=1)


def _perm_gates_ifog(w):
    # torch gate order i,f,g,o (blocks of H) -> our chunk order i,f,o,g;
    # g rows scaled by 2 for the sigma-trick (tanh(g) = 2*sigma(2g) - 1)
    i, f, g, o = np.split(w, 4, axis=0)
    return np.concatenate([i, f, o, 2.0 * g], axis=0)


def prepare_in_maps(**inputs):
    x = np.asarray(inputs["x"]).astype(np.int32)          # [32, 256]
    tags = np.asarray(inputs["tags"]).astype(np.int32)
    emb = np.asarray(inputs["emb"], dtype=np.float32)
    lin_w = np.asarray(inputs["lin_w"], dtype=np.float32)
    lin_b = np.asarray(inputs["lin_b"], dtype=np.float32)
    start_t = np.asarray(inputs["start_t"], dtype=np.float32)
    end_t = np.asarray(inputs["end_t"], dtype=np.float32)
    trans = np.asarray(inputs["trans"], dtype=np.float32)

    wihp = {0: _perm_gates_ifog(np.asarray(inputs["w_ih_f"], np.float32)),
            1: _perm_gates_ifog(np.asarray(inputs["w_ih_b"], np.float32))}
    whhp = {0: _perm_gates_ifog(np.asarray(inputs["w_hh_f"], np.float32)),
            1: _perm_gates_ifog(np.asarray(inputs["w_hh_b"], np.float32))}
    bp = {0: _perm_gates_ifog(np.asarray(inputs["b_f"], np.float32).reshape(-1, 1)),
          1: _perm_gates_ifog(np.asarray(inputs["b_b"], np.float32).reshape(-1, 1))}

    wih_t = {dd: _pack_wT(wihp[dd], 2).astype(ml_dtypes.float8_e4m3) for dd in (0, 1)}
    whh_t = {dd: _pack_wT(whhp[dd], 4).astype(ml_dtypes.float8_e4m3) for dd in (0, 1)}

    # biasbc [128, 128]: col = d*64 + m*4 + s -> b_d[m*128 + p]
    biasbc = np.zeros((128, 128), np.float32)
    for dd in (0, 1):
        for m in range(16):
            col = bp[dd][m * 128:(m + 1) * 128, 0]
            for s in range(4):
                biasbc[:, dd * 64 + 4 * m + s] = col
    biasbc = biasbc.astype(ml_dtypes.bfloat16)

    # linT [128, 8*T]: tile kc = lin_w[:, kc*128:(kc+1)*128].T (fwd 0-3, bwd 4-7)
    lin_tiles = [np.ascontiguousarray(lin_w[:, kc * 128:(kc + 1) * 128].T)
                 for kc in range(8)]
    linT = np.concatenate(lin_tiles, axis=1).astype(ml_dtypes.bfloat16)

    id128 = np.eye(128, dtype=np.float32)

    # stacked (2 seqs per tile, second at partition 64) CRF operands:
    # epstk = block-diag(Ep, Ep), epstkT = block-diag(Ep^T, Ep^T) with
    # Ep = exp(trans - ln45); i45stk = block-diag(I45, I45);
    # stendstk = [start_t | end_t] stacked.
    Ep = np.exp(np.float64(trans) - np.log(45.0)).astype(np.float32)
    epstk = np.zeros((128, 128), np.float32)
    epstkT = np.zeros((128, 128), np.float32)
    i45stk = np.zeros((128, 45), np.float32)
    for r0 in (0, 64):
        epstk[r0:r0 + 45, r0:r0 + 45] = Ep
        epstkT[r0:r0 + 45, r0:r0 + 45] = Ep.T
        i45stk[r0:r0 + 45, :] = np.eye(45)
    epstk = epstk.astype(ml_dtypes.bfloat16)
    epstkT = epstkT.astype(ml_dtypes.bfloat16)
    i45stk = i45stk.astype(ml_dtypes.bfloat16)
    stendstk = np.zeros((128, 2), np.float32)
    linbstk = np.zeros((128, 1), np.float32)
    for r0 in (0, 64):
        stendstk[r0:r0 + 45, 0] = start_t
        stendstk[r0:r0 + 45, 1] = end_t
        linbstk[r0:r0 + 45, 0] = lin_b

    in_maps = []
    for core in range(8):
        seqs = slice(4 * core, 4 * core + 4)
        xs = x[seqs]                                      # [4, 256]
        # xidx [128, 8]: col b, row r -> x[s=(r%4), t=(128b+r)//4]
        nflat = xs.T.reshape(-1)                          # n = 4t+s
        xidx = np.ascontiguousarray(nflat.reshape(8, 128).T).astype(np.int32)

        tg = tags[seqs]                                   # [4, 256]
        oh = np.zeros((T, N), np.float32)
        oh[tg.T.reshape(-1), np.arange(N)] = 1.0
        oh2 = np.zeros((T, N), np.float32)
        oh2[:, 0:N - NS] = oh[:, NS:N]

        in_maps.append({
            "emb": emb.astype(ml_dtypes.bfloat16),
            "xidx": xidx,
            "wihf": wih_t[0], "wihb": wih_t[1],
            "whhf": whh_t[0], "whhb": whh_t[1],
            "biasbc": biasbc,
            "linT": linT,
            "linb": lin_b.reshape(T, 1),
            "id128": id128,
            "idbf": np.eye(128, dtype=ml_dtypes.bfloat16),
            "trans": trans,
            "stend": np.stack([start_t, end_t], axis=1),
            "epstk": epstk,
            "epstkT": epstkT,
            "i45stk": i45stk,
            "stendstk": stendstk,
            "linbstk": linbstk,
            "oh": oh,
            "oh2": oh2,
        })
    return in_maps


def get_nc():
    if "nc" not in _cached:
        _cached["nc"] = _build()
    return _cached["nc"]


def kernel(**inputs):
    in_maps = prepare_in_maps(**inputs)
    res = run_bass_kernel_spmd(get_nc(), in_maps, core_ids=list(range(8)))
    total = np.float64(0.0)
    for core in range(8):
        total += np.float64(res.results[core]["loss"]).sum()
    return np.float32(total / 32.0)


# revision 58
# speedup vs baseline: 1.0447x; 1.0447x over previous
"""BiLSTM-CRF NLL loss on 8 Trainium2 NeuronCores.

Sharding: core c owns sequences [4c, 4c+4); each core runs BOTH LSTM
directions and the full CRF for its 4 sequences. No collectives; host sums
8 per-core partials.

Recurrence: TWO phase-shifted chains per core, chain ch owns seqs
{2ch, 2ch+1} and carries BOTH directions in merged instructions. Per
chain-step: one PSUM tile G [128, 64] (cols = d*32 + m*2 + s'), one
accumulation group: bias-inject identity mm (start=True) + 32 x-mms +
64 h-mms (stop on last). sigma-trick: g-gate rows of W/b pre-scaled by 2 so
ONE Sigmoid activation covers all 16 gate chunks (tanh(g) = 2*sigma(2g)-1);
cell update is 4 fused DVE ops (c = t1 + 2*i*s - i), then tanh(c) Act and
the h-mul DVE writes both directions' h slots via a strided AP. This cuts
the per-step serial chain to PE -> Act -> DVE -> Act -> DVE with two
independent chains pipelined to hide the cross-engine latency.

Weights fp8e4m3, x/h matmuls fp8 DoubleRow. h stored fp8 unit-major.

CRF partition function: exp-domain, two-sided (alpha ascends, beta descends,
meet at K=127), bf16 chain operands. Gold-path score via one-hot matmuls.
Loss exits as [1,4] per-core partials.

Self-contained: hardcodes all shapes; only needs numpy + concourse (+ml_dtypes).
"""
import numpy as np
import ml_dtypes

import concourse.bass as bass
import concourse.bacc as bacc
import concourse.tile as tile
from concourse import mybir
from concourse.tile_rust import add_dep_helper
from concourse.bass_utils import run_bass_kernel_spmd

F32 = mybir.dt.float32
FP8 = mybir.dt.float8e4
BF16 = mybir.dt.bfloat16
I32 = mybir.dt.int32
AF = mybir.ActivationFunctionType
ALU = mybir.AluOpType

B, S, E, H, T, V = 32, 256, 256, 512, 45, 50000
NS = 4                 # seqs per core
N = S * NS             # 1024 emission cols, n = 4t+s
NCH = 16               # gate chunks (2048/128)
HC = 4                 # h chunks (512/128)
SW = HC * NS           # state cols per step = 16 (all 4 seqs)
NB_T = 4               # transform n-blocks (of 64 steps = 256 cols each)
TBLK = S // NB_T       # 64 steps per transform block
LN45 = float(np.log(45.0))
DSTRIDE = (S + 1) * SW  # 4112: cols per direction in hsT_all

_cached = {}


def _build(stop_after=None):
    lv = {"xf": 1, "rec": 2, "em": 3, "crf": 4, None: 5}[stop_after]
    nc = bacc.Bacc("TRN2", target_bir_lowering=False, debug=False, num_devices=8)

    d = {}
    d["emb"] = nc.dram_tensor("emb", [V, E], BF16, kind="ExternalInput")
    d["xidx"] = nc.dram_tensor("xidx", [128, 8], I32, kind="ExternalInput")
    d["wihf"] = nc.dram_tensor("wihf", [128, 32 * 128], FP8, kind="ExternalInput")
    d["wihb"] = nc.dram_tensor("wihb", [128, 32 * 128], FP8, kind="ExternalInput")
    d["whhf"] = nc.dram_tensor("whhf", [128, 64 * 128], FP8, kind="ExternalInput")
    d["whhb"] = nc.dram_tensor("whhb", [128, 64 * 128], FP8, kind="ExternalInput")
    d["biasbc"] = nc.dram_tensor("biasbc", [128, 128], BF16, kind="ExternalInput")
    d["linT"] = nc.dram_tensor("linT", [128, 8 * 64], BF16, kind="ExternalInput")
    d["linb"] = nc.dram_tensor("linb", [T, 1], F32, kind="ExternalInput")
    d["id128"] = nc.dram_tensor("id128", [128, 128], F32, kind="ExternalInput")
    d["idbf"] = nc.dram_tensor("idbf", [128, 128], BF16, kind="ExternalInput")
    d["trans"] = nc.dram_tensor("trans", [T, T], F32, kind="ExternalInput")
    d["stend"] = nc.dram_tensor("stend", [T, 2], F32, kind="ExternalInput")
    d["epstk"] = nc.dram_tensor("epstk", [128, 128], BF16, kind="ExternalInput")
    d["epstkT"] = nc.dram_tensor("epstkT", [128, 128], BF16, kind="ExternalInput")
    d["i45stk"] = nc.dram_tensor("i45stk", [128, 45], BF16, kind="ExternalInput")
    d["stendstk"] = nc.dram_tensor("stendstk", [128, 2], F32, kind="ExternalInput")
    d["linbstk"] = nc.dram_tensor("linbstk", [128, 1], F32, kind="ExternalInput")
    d["oh"] = nc.dram_tensor("oh", [T, N], F32, kind="ExternalInput")
    d["oh2"] = nc.dram_tensor("oh2", [T, N], F32, kind="ExternalInput")
    d_loss = nc.dram_tensor("loss", [1, NS], F32, kind="ExternalOutput")

    with tile.TileContext(nc) as tc:
        with tc.tile_pool(name="persist", bufs=1) as pp, \
             tc.tile_pool(name="gxp", bufs=1) as gxp:
            # persistent weights / tables
            wih = {0: pp.tile([128, 32 * 128], FP8, tag="wihf", name="wihf"),
                   1: pp.tile([128, 32 * 128], FP8, tag="wihb", name="wihb")}
            whh = {0: pp.tile([128, 64 * 128], FP8, tag="whhf", name="whhf"),
                   1: pp.tile([128, 64 * 128], FP8, tag="whhb", name="whhb")}
            biasbc = pp.tile([128, 128], BF16, tag="biasbc")
            ones1 = pp.tile([1, NS], F32, tag="ones1")
            id128 = pp.tile([128, 128], F32, tag="id128")
            idbf = pp.tile([128, 128], BF16, tag="idbf")
            xidx = pp.tile([128, 8], I32, tag="xidx")
            linT = pp.tile([128, 8 * 64], BF16, tag="linT")
            linb = pp.tile([T, 1], F32, tag="linb")
            epstk = pp.tile([128, 128], BF16, tag="epstk")
            epstkT = pp.tile([128, 128], BF16, tag="epstkT")
            i45stk = pp.tile([128, 45], BF16, tag="i45stk")
            stendstk = pp.tile([128, 2], F32, tag="stendstk")
            # estk[pair]: exp(emissions), seq-pair stacked on partitions
            # (second seq at partition 64), one column per position
            estk = {0: pp.tile([128, S], F32, tag="estk0", name="estk0"),
                    1: pp.tile([128, S], F32, tag="estk1", name="estk1")}
            em_lin = pp.tile([T, N], F32, tag="em_lin")
            trans_sb = pp.tile([T, T], F32, tag="trans")
            stend = pp.tile([T, 2], F32, tag="stend")
            oh = pp.tile([T, N], F32, tag="oh")
            oh2 = pp.tile([T, N], F32, tag="oh2")
            nc.sync.dma_start(out=xidx[:], in_=d["xidx"][:])
            nc.sync.dma_start(out=id128[:], in_=d["id128"][:])
            nc.sync.dma_start(out=idbf[:], in_=d["idbf"][:])
            nc.sync.dma_start(out=biasbc[:], in_=d["biasbc"][:])
            nc.sync.dma_start(out=linb[:], in_=d["linb"][:])
            nc.sync.dma_start(out=epstk[:], in_=d["epstk"][:])
            nc.sync.dma_start(out=epstkT[:], in_=d["epstkT"][:])
            nc.sync.dma_start(out=i45stk[:], in_=d["i45stk"][:])
            nc.sync.dma_start(out=stendstk[:], in_=d["stendstk"][:])
            nc.sync.dma_start(out=trans_sb[:], in_=d["trans"][:])
            nc.sync.dma_start(out=stend[:], in_=d["stend"][:])
            nc.sync.dma_start(out=oh[:], in_=d["oh"][:])
            nc.sync.dma_start(out=oh2[:], in_=d["oh2"][:])
            nc.vector.memset(ones1[:], 1.0)

            # XT block tiles: [nb] -> [128, 2 ec x 256 n] fp8
            xt = {nb: gxp.tile([128, 2 * TBLK * NS], FP8, tag=f"xt{nb}", name=f"xt{nb}")
                  for nb in range(NB_T)}
            # h state, both dirs in one tile (unit-major):
            # col = d*DSTRIDE + 16*slot + 4*k + s ; slot S = zeros (h0).
            # BOTH directions write slot u at recurrence step u: the bwd
            # direction's h (position S-1-u) is stored TIME-REVERSED, so
            # h-mm reads and the h-write are uniform across dirs. The
            # reversal is undone in the emissions phase via a reversed AP.
            hsT = pp.tile([128, 2 * DSTRIDE], FP8, tag="hsT", name="hsT")
            nc.vector.memset(hsT[:, SW * S: SW * S + SW], 0.0)
            nc.vector.memset(hsT[:, DSTRIDE + SW * S: DSTRIDE + SW * S + SW], 0.0)

            # ---------- phase 0: gather + transpose -> XT ----------
            with tc.tile_pool(name="gat", bufs=3) as gp, \
                 tc.tile_pool(name="ps_tp", bufs=4, space="PSUM") as ps_tp:
                for b in range(8):
                    X = gp.tile([128, E], BF16, tag="X")
                    nc.gpsimd.indirect_dma_start(
                        out=X[:],
                        out_offset=None,
                        in_=d["emb"][:],
                        in_offset=bass.IndirectOffsetOnAxis(ap=xidx[:, b:b + 1], axis=0),
                    )
                    nb, off = b // 2, (b % 2) * 128
                    for ec in range(2):
                        tp = ps_tp.tile([128, 128], BF16, tag="tp")
                        nc.tensor.transpose(tp[:], X[:, 128 * ec: 128 * ec + 128], idbf[:])
                        nc.vector.tensor_copy(
                            xt[nb][:, TBLK * NS * ec + off: TBLK * NS * ec + off + 128],
                            tp[:])

            # weight DMAs after the gathers so they share the DMA engines
            nc.sync.dma_start(out=wih[0][:], in_=d["wihf"][:])
            nc.sync.dma_start(out=wih[1][:], in_=d["wihb"][:])
            nc.sync.dma_start(out=whh[0][:], in_=d["whhf"][:])
            nc.sync.dma_start(out=whh[1][:], in_=d["whhb"][:])
            nc.sync.dma_start(out=linT[:], in_=d["linT"][:])

            # ---------- recurrence ----------
            if lv == 1:
                probe = pp.tile([1, NS], F32, tag="probe")
                nc.vector.tensor_copy(probe[:], xt[0][0:1, 0:NS])
                nc.sync.dma_start(out=d_loss[:], in_=probe[:])
            zres = pp.tile([1, NS], F32, tag="zres")      # sum_j u1*v1 per seq
            em_stk = {0: pp.tile([128, S], F32, tag="emstk0", name="emstk0"),
                      1: pp.tile([128, S], F32, tag="emstk1", name="emstk1")}
            linbstk = pp.tile([128, 1], F32, tag="linbstk")
            nc.sync.dma_start(out=linbstk[:], in_=d["linbstk"][:])
            if lv >= 2:
                cfp = tc.alloc_tile_pool(name="crfp", bufs=3)
                cur = {}
                with tc.tile_pool(name="rec0", bufs=6) as rp0, \
                     tc.tile_pool(name="rec1", bufs=6) as rp1, \
                     tc.tile_pool(name="psg0", bufs=2, space="PSUM") as pg0, \
                     tc.tile_pool(name="psg1", bufs=2, space="PSUM") as pg1, \
                     tc.tile_pool(name="psem", bufs=1, space="PSUM") as ps_em, \
                     tc.tile_pool(name="pscrf", bufs=1, space="PSUM") as ps_crf:
                    rp = [rp0, rp1]
                    pg = [pg0, pg1]
                    cprev = [None, None]
                    for ch in (0, 1):
                        cinit = rp[ch].tile([128, 16], BF16, tag="c")
                        nc.vector.memset(cinit[:], 0.0)
                        cprev[ch] = cinit

                    DR = mybir.MatmulPerfMode.DoubleRow
                    xtv = {nb: xt[nb].rearrange("p (e c) -> p e c", e=2)
                           for nb in range(NB_T)}
                    hv = hsT.rearrange("p (a t k s) -> p a t k s",
                                       a=2, t=S + 1, k=HC, s=NS)
                    def stage_pe(dd, u):
                        # chain dd = ONE direction, all 4 seqs. Both dirs
                        # write slot u, read slot u-1 (u=0: slot S = zeros);
                        # bwd h is stored time-reversed.
                        slot_r = S if u == 0 else u - 1
                        col = u if dd == 0 else S - 1 - u   # xt timestep

                        G = pg[dd].tile([128, 64], F32, tag="G")
                        # one accumulation group per G tile: bias-inject
                        # (start=True zeroes the region) -> x-mms -> h-mms
                        # (stop=True on the very last; at u=0 there are no
                        # h-mms so the last x-mm stops). add_dep_helper pins
                        # start-first / stop-last against scheduler
                        # reordering. Phase A (inject + x) has no h dep and
                        # runs in the previous step's tail.
                        mms = []
                        mm = nc.tensor.matmul(
                            G[:], idbf[:],
                            biasbc[:, dd * 64: dd * 64 + 64],
                            start=True, stop=False)
                        mms.append(mm)
                        nb = col // TBLK
                        j = col % TBLK
                        for m in range(NCH):
                            sl = G[:, 4 * m: 4 * m + 4]
                            wpair = wih[dd][:, 2 * m * 128:(2 * m + 2) * 128]
                            mm = nc.tensor.matmul(
                                sl,
                                wpair.rearrange("p (c f) -> p c f", c=2),
                                xtv[nb][:, :, NS * j: NS * j + NS],
                                start=False,
                                stop=(u == 0 and m == NCH - 1),
                                perf_mode=DR)
                            mms.append(mm)
                        if u > 0:
                            for m in range(NCH):
                                sl = G[:, 4 * m: 4 * m + 4]
                                for kp in range(HC // 2):
                                    hpair = whh[dd][
                                        :, (4 * m + 2 * kp) * 128:
                                        (4 * m + 2 * kp + 2) * 128]
                                    rhs = hv[:, dd, slot_r,
                                             2 * kp: 2 * kp + 2, :]
                                    mm = nc.tensor.matmul(
                                        sl,
                                        hpair.rearrange(
                                            "p (c f) -> p c f", c=2),
                                        rhs,
                                        start=False,
                                        stop=(m == NCH - 1
                                              and kp == HC // 2 - 1),
                                        perf_mode=DR)
                                    mms.append(mm)
                        first, last = mms[0], mms[-1]
                        for mm in mms[1:]:
                            add_dep_helper(mm.ins, first.ins, sync=False,
                                           reason="group start first")
                        for mm in mms[:-1]:
                            add_dep_helper(last.ins, mm.ins, sync=False,
                                           reason="group stop last")
                        return G

                    def stage_sig(dd, G):
                        # T = tanh over ALL 64 gate cols. Weights carry a
                        # global x8 encoding scale (fp8 normal range),
                        # undone here; i/f/o rows also carry the x0.5 of
                        # sigma(x) = (1+tanh(x/2))/2, and the h-state is
                        # h~ = 2h with the compensation folded into W_hh
                        # and lin_w. Only tanh/exp/copy Act funcs are used
                        # anywhere in the loop -> one act table, no reloads.
                        SG = rp[dd].tile([128, 64], BF16, tag="SG")
                        nc.scalar.activation(SG[:], G[:], AF.Tanh,
                                             scale=0.125)
                        return SG

                    def stage_cell(dd, SG):
                        SGv = SG.rearrange("p (m s) -> p m s", m=NCH)
                        ihat = SGv[:, 0:4, :]
                        fhat = SGv[:, 4:8, :]
                        shat = SGv[:, 12:16, :]
                        t1 = rp[dd].tile([128, 16], BF16, tag="t1")
                        t2 = rp[dd].tile([128, 16], BF16, tag="t2")
                        cnext = rp[dd].tile([128, 16], BF16, tag="c")
                        cpv = cprev[dd].rearrange("p (k s) -> p k s", k=4)
                        t1v = t1.rearrange("p (k s) -> p k s", k=4)
                        t2v = t2.rearrange("p (k s) -> p k s", k=4)
                        cnv = cnext.rearrange("p (k s) -> p k s", k=4)
                        # state c~ = 2c:
                        # c~ = (1+T_f)/2 * c~_prev + (1+T_i)*T_g
                        nc.vector.scalar_tensor_tensor(
                            out=t1v[:], in0=fhat, scalar=1.0, in1=cpv[:],
                            op0=ALU.add, op1=ALU.mult)
                        nc.vector.scalar_tensor_tensor(
                            out=t2v[:], in0=ihat, scalar=1.0, in1=shat,
                            op0=ALU.add, op1=ALU.mult)
                        nc.vector.scalar_tensor_tensor(
                            out=cnv[:], in0=t1v[:], scalar=0.5, in1=t2v[:],
                            op0=ALU.mult, op1=ALU.add)
                        cprev[dd] = cnext
                        return cnext

                    def stage_tanh(dd, cnext):
                        # th = tanh(c) = tanh(0.5 * c~)
                        th = rp[dd].tile([128, 16], BF16, tag="th")
                        nc.scalar.activation(th[:], cnext[:], AF.Tanh,
                                             scale=0.5)
                        return th

                    def stage_h(dd, u, SG, th):
                        SGv = SG.rearrange("p (m s) -> p m s", m=NCH)
                        ohat = SGv[:, 8:12, :]
                        # h~ = 2h = (1+T_o)*th
                        hout = hv[:, dd, u, :, :]
                        thv = th.rearrange("p (k s) -> p k s", k=4)
                        nc.vector.scalar_tensor_tensor(
                            out=hout, in0=ohat, scalar=1.0, in1=thv[:],
                            op0=ALU.add, op1=ALU.mult)

                    # ---- incremental emissions + mid-out CRF machinery ----
                    # At step u>=128, positions u and S-1-u complete. Every
                    # 8 steps two 8-position blocks (A: ascending from 128,
                    # B: descending from 127) get emissions computed into
                    # the seq-pair-stacked em_stk/estk layout. The CRF
                    # partition function is two matrix-product chains per
                    # seq pair (R consumes t=128..255 ascending, L~ consumes
                    # t=127..1 descending), one link per chain per step,
                    # hidden behind the recurrence. Sub-stages are emitted
                    # in data-arrival order per engine stream.
                    emctx = {}

                    def emit_block_pe(key, p0, fslot0, bslot0):
                        # emissions for positions [p0, p0+8); bwd slots
                        # [bslot0, bslot0+8) ascending = positions reversed.
                        # One PSUM tile per seq PAIR (partitions 0-44):
                        # seq-half sh at cols 16*sh (+0-7 fwd, +8-15 bwd),
                        # one accumulation group.
                        pes = []
                        for pr in (0, 1):
                            pe = ps_em.tile([T, 32], F32, tag=f"pe{pr}",
                                            name=f"pe{pr}")
                            emms = []
                            for sh in (0, 1):
                                seq = 2 * pr + sh
                                for c0_, dd, sl0, kb in ((0, 0, fslot0, 0),
                                                         (8, 1, bslot0, 4)):
                                    for k in range(4):
                                        kc = kb + k
                                        cb = 16 * sh + c0_
                                        emms.append(nc.tensor.matmul(
                                            pe[:, cb: cb + 8],
                                            linT[:, 64 * kc: 64 * kc + T],
                                            hv[:, dd, sl0:sl0 + 8, k, seq],
                                            start=(sh == 0 and c0_ == 0
                                                   and k == 0),
                                            stop=(sh == 1 and c0_ == 8
                                                  and k == 3)))
                            first, last = emms[0], emms[-1]
                            for m_ in emms[1:]:
                                add_dep_helper(m_.ins, first.ins, sync=False,
                                               reason="em start first")
                            for m_ in emms[:-1]:
                                add_dep_helper(last.ins, m_.ins, sync=False,
                                               reason="em stop last")
                            pes.append(pe)
                        emctx[key] = (p0, pes)

                    def emit_block_dve(key):
                        # combine fwd + reversed bwd into em_lin directly,
                        # then stack into em_stk (DMA shifts partitions for
                        # the odd seq of each pair; DVE copies the even seq)
                        p0, pes = emctx[key]
                        emv = em_lin.rearrange("p (t s) -> p t s", s=NS)
                        for seq in range(NS):
                            pr, sh = seq // 2, seq % 2
                            pb_s = cfp.tile([T, 8], F32, tag=f"pbs{seq}",
                                            name=f"pbs{seq}")
                            nc.vector.tensor_copy(
                                pb_s[:], pes[pr][:, 16 * sh + 8: 16 * sh + 16])
                            nc.vector.tensor_tensor(
                                out=emv[:, p0:p0 + 8, seq],
                                in0=pes[pr][:, 16 * sh: 16 * sh + 8],
                                in1=pb_s[:, ::-1], op=ALU.add)
                        for pr in (0, 1):
                            nc.vector.tensor_copy(
                                em_stk[pr][0:T, p0:p0 + 8],
                                emv[:, p0:p0 + 8, 2 * pr])
                            nc.sync.dma_start(
                                out=em_stk[pr][64:64 + T, p0:p0 + 8],
                                in_=emv[:, p0:p0 + 8, 2 * pr + 1])

                    act_pending = []

                    def emit_block_act(item):
                        key, (p0, pes) = item
                        for pr in (0, 1):
                            nc.scalar.activation(
                                estk[pr][:, p0:p0 + 8],
                                em_stk[pr][:, p0:p0 + 8], AF.Exp,
                                bias=linbstk[:])

                    cur.update({("L", 0): i45stk, ("L", 1): i45stk,
                                ("R", 0): i45stk, ("R", 1): i45stk})

                    def link_L_rs(pr, t):
                        rs = cfp.tile([128, T], BF16, tag=f"rL{pr}",
                                      name=f"rL{pr}")
                        nc.vector.tensor_scalar_mul(
                            rs[:], cur[("L", pr)][:], estk[pr][:, t:t + 1])
                        return rs

                    def link_L_mm(pr, rs):
                        ps = ps_crf.tile([128, T], F32, tag="psL",
                                         name="psL")
                        nc.tensor.matmul(ps[:], epstkT[:], rs[:],
                                         start=True, stop=True)
                        return ps

                    def link_L_fin(pr, ps):
                        nxt = cfp.tile([128, T], BF16, tag=f"cL{pr}",
                                       name=f"cL{pr}")
                        nc.vector.tensor_copy(nxt[:], ps[:])
                        cur[("L", pr)] = nxt

                    def link_R_mm(pr):
                        ps = ps_crf.tile([128, T], F32, tag="psR",
                                         name="psR")
                        nc.tensor.matmul(ps[:], epstk[:], cur[("R", pr)][:],
                                         start=True, stop=True)
                        return ps

                    def link_R_fin(pr, t, ps):
                        nxt = cfp.tile([128, T], BF16, tag=f"cR{pr}",
                                       name=f"cR{pr}")
                        nc.scalar.activation(nxt[:], ps[:], AF.Copy,
                                             scale=estk[pr][:, t:t + 1])
                        cur[("R", pr)] = nxt

                    def links_tail(r_t, l_t):
                        # one position per chain, engine-ordered
                        rsl = {}
                        if l_t is not None:
                            rsl[0] = link_L_rs(0, l_t)
                            rsl[1] = link_L_rs(1, l_t)
                        psr = {}
                        if r_t is not None:
                            psr[0] = link_R_mm(0)
                            psr[1] = link_R_mm(1)
                        psl = {}
                        if l_t is not None:
                            psl[0] = link_L_mm(0, rsl[0])
                            psl[1] = link_L_mm(1, rsl[1])
                        if r_t is not None:
                            link_R_fin(0, r_t, psr[0])
                            link_R_fin(1, r_t, psr[1])
                        if l_t is not None:
                            link_L_fin(0, psl[0])
                            link_L_fin(1, psl[1])

                    # stage-major emission: each engine's stream is ordered
                    # by expected data-arrival time so the in-order engine
                    # FIFOs never head-block across the two chains.
                    r_next, r_avail = [128], [127]
                    l_next, l_avail = [127], [128]

                    for u in range(S):
                        G0 = stage_pe(0, u)
                        G1 = stage_pe(1, u)
                        SG0 = stage_sig(0, G0)
                        SG1 = stage_sig(1, G1)
                        r_t = l_t = None
                        if lv >= 3:
                            if r_next[0] <= r_avail[0]:
                                r_t = r_next[0]
                                r_next[0] += 1
                            if l_next[0] >= max(l_avail[0], 1):
                                l_t = l_next[0]
                                l_next[0] -= 1
                        rsl = {}
                        if l_t is not None:
                            rsl[0] = link_L_rs(0, l_t)
                            rsl[1] = link_L_rs(1, l_t)
                        c0 = stage_cell(0, SG0)
                        c1 = stage_cell(1, SG1)
                        th0 = stage_tanh(0, c0)
                        th1 = stage_tanh(1, c1)
                        stage_h(0, u, SG0, th0)
                        stage_h(1, u, SG1, th1)
                        # emission block AFTER the h-writes it reads
                        isem = lv >= 3 and u >= 135 and u % 8 == 7
                        if isem:
                            b = (u - 135) // 8
                            emit_block_pe("A", 128 + 8 * b, 128 + 8 * b,
                                          120 - 8 * b)
                            emit_block_pe("B", 120 - 8 * b, 120 - 8 * b,
                                          128 + 8 * b)
                            emit_block_dve("A")
                            emit_block_dve("B")
                            for key in act_pending:
                                emit_block_act(key)
                            act_pending.clear()
                            act_pending.append(("A", emctx["A"]))
                            act_pending.append(("B", emctx["B"]))
                        psr = {}
                        if r_t is not None:
                            psr[0] = link_R_mm(0)
                            psr[1] = link_R_mm(1)
                        psl = {}
                        if l_t is not None:
                            psl[0] = link_L_mm(0, rsl[0])
                            psl[1] = link_L_mm(1, rsl[1])
                        if r_t is not None:
                            link_R_fin(0, r_t, psr[0])
                            link_R_fin(1, r_t, psr[1])
                        if l_t is not None:
                            link_L_fin(0, psl[0])
                            link_L_fin(1, psl[1])
                        if isem:
                            # commit the PREVIOUS block's positions: one
                            # extra block of lag so the stacking DMA + exp
                            # finish before links consume them (keeps the
                            # DVE stream from head-blocking on estk)
                            b = (u - 135) // 8
                            if b >= 1:
                                r_avail[0] = 135 + 8 * (b - 1)
                                l_avail[0] = 120 - 8 * (b - 1)

                    if lv >= 3:
                        for key in act_pending:
                            emit_block_act(key)
                        act_pending.clear()
                        # drain remaining links after the recurrence
                        while r_next[0] <= 255 or l_next[0] >= 1:
                            r_t = l_t = None
                            if r_next[0] <= 255:
                                r_t = r_next[0]
                                r_next[0] += 1
                            if l_next[0] >= 1:
                                l_t = l_next[0]
                                l_next[0] -= 1
                            links_tail(r_t, l_t)

                if lv >= 3:
                    with tc.tile_pool(name="psfin", bufs=1,
                                      space="PSUM") as ps_fin:
                        # Z = eend^T R L alpha0, alpha0 = exp(start) * e_0
                        esstk = cfp.tile([128, 2], BF16, tag="esstk")
                        nc.scalar.activation(esstk[:], stendstk[:], AF.Exp)
                        v1p = ps_fin.tile([T, NS], F32, tag="v1p")
                        u1p = ps_fin.tile([T, NS], F32, tag="u1p")
                        fmms_v, fmms_u = [], []
                        a0 = {}
                        for pr in (0, 1):
                            a0[pr] = cfp.tile([128, 1], BF16, tag=f"a0{pr}",
                                              name=f"a0{pr}")
                            nc.vector.tensor_mul(a0[pr][:], esstk[:, 0:1],
                                                 estk[pr][:, 0:1])
                        for s in range(NS):
                            pr, sh = s // 2, s % 2
                            r0 = 64 * sh
                            fmms_v.append(nc.tensor.matmul(
                                v1p[:, s:s + 1],
                                cur[("L", pr)][r0:r0 + T, :],
                                a0[pr][r0:r0 + T, :],
                                start=(s == 0), stop=(s == NS - 1)))
                            fmms_u.append(nc.tensor.matmul(
                                u1p[:, s:s + 1],
                                cur[("R", pr)][r0:r0 + T, :],
                                esstk[r0:r0 + T, 1:2],
                                start=(s == 0), stop=(s == NS - 1)))
                        for fm in (fmms_v, fmms_u):
                            for m_ in fm[1:]:
                                add_dep_helper(m_.ins, fm[0].ins, sync=False,
                                               reason="z start first")
                            for m_ in fm[:-1]:
                                add_dep_helper(fm[-1].ins, m_.ins, sync=False,
                                               reason="z stop last")
                        v1s = cfp.tile([T, NS], F32, tag="v1s")
                        wz = cfp.tile([T, NS], F32, tag="wz")
                        nc.vector.tensor_copy(v1s[:], v1p[:])
                        nc.vector.tensor_mul(wz[:], v1s[:], u1p[:])
                        zp = ps_fin.tile([1, NS], F32, tag="zp")
                        ones45r = cfp.tile([T, 1], F32, tag="ones45r")
                        nc.vector.memset(ones45r[:], 1.0)
                        nc.tensor.matmul(zp[:], ones45r[:], wz[:],
                                         start=True, stop=True)
                        nc.vector.tensor_copy(zres[:], zp[:])

                        # de-stack em into em_lin [T, 4t+s] for gold score
                        for pr in (0, 1):
                            for sh in (0, 1):
                                seq = 2 * pr + sh
                                emv = em_lin.rearrange("p (t s) -> p t s",
                                                       s=NS)
                                nc.sync.dma_start(
                                    out=emv[:, :, seq],
                                    in_=em_stk[pr][64 * sh: 64 * sh + T, :])

                if lv == 2:
                    probe = pp.tile([1, NS], F32, tag="probe")
                    nc.vector.tensor_copy(probe[:], hsT[0:1, 0:NS])
                    nc.sync.dma_start(out=d_loss[:], in_=probe[:])

            if lv >= 2:
                cfp.release()

            # ---------- gold score + final loss ----------
            if lv == 3:
                probe = pp.tile([1, NS], F32, tag="probe")
                nc.vector.tensor_copy(probe[:], em_lin[0:1, 0:NS])
                nc.sync.dma_start(out=d_loss[:], in_=probe[:])
            if lv >= 4:
                with tc.tile_pool(name="crf", bufs=1) as cp, \
                     tc.tile_pool(name="ps_f", bufs=1, space="PSUM") as ps_f:
                    ones45 = cp.tile([T, 1], F32, tag="ones45")
                    ones45b = cp.tile([T, 1], BF16, tag="ones45b")
                    nc.vector.memset(ones45[:], 1.0)
                    nc.vector.memset(ones45b[:], 1.0)

                    logZ = cp.tile([1, NS], F32, tag="logZ")
                    em_h = cp.tile([1, 2 * NS], F32, tag="em_h")
                    tr_h = cp.tile([1, 2 * NS], F32, tag="tr_h")
                    em_sc = cp.tile([1, NS], F32, tag="em_sc")
                    tr_sc = cp.tile([1, NS], F32, tag="tr_sc")
                    sten_s = cp.tile([1, NS], F32, tag="sten_s")
                    nc.scalar.activation(logZ[:], zres[:], AF.Ln)

                    # S1 = (em_lin + linb) * onehot(tags)
                    S1 = cp.tile([T, N], BF16, tag="S1")
                    nc.vector.scalar_tensor_tensor(
                        out=S1[:], in0=em_lin[:], scalar=linb[:], in1=oh[:],
                        op0=ALU.add, op1=ALU.mult)
                    S2 = cp.tile([T, N], BF16, tag="S2")
                    for ck in range(2):
                        sl = slice(512 * ck, 512 * (ck + 1))
                        s1p = ps_f.tile([1, 512], F32, tag="fbig")
                        nc.tensor.matmul(s1p[:], ones45b[:], S1[:, sl],
                                         start=True, stop=True)
                        nc.vector.tensor_reduce(
                            em_h[:, NS * ck: NS * (ck + 1)],
                            s1p.rearrange("p (t b) -> p b t", b=NS),
                            axis=mybir.AxisListType.X, op=ALU.add)
                        Rp_ = ps_f.tile([T, 512], F32, tag="fR")
                        nc.tensor.matmul(Rp_[:], trans_sb[:], oh[:, sl],
                                         start=True, stop=True)
                        nc.vector.tensor_mul(S2[:, sl], Rp_[:], oh2[:, sl])
                        s2p = ps_f.tile([1, 512], F32, tag="fbig2")
                        nc.tensor.matmul(s2p[:], ones45b[:], S2[:, sl],
                                         start=True, stop=True)
                        nc.vector.tensor_reduce(
                            tr_h[:, NS * ck: NS * (ck + 1)],
                            s2p.rearrange("p (t b) -> p b t", b=NS),
                            axis=mybir.AxisListType.X, op=ALU.add)
                    nc.vector.tensor_add(em_sc[:], em_h[:, 0:NS],
                                         em_h[:, NS:2 * NS])
                    nc.vector.tensor_add(tr_sc[:], tr_h[:, 0:NS],
                                         tr_h[:, NS:2 * NS])

                    stp = cp.tile([T, NS], F32, tag="stp")
                    enp = cp.tile([T, NS], F32, tag="enp")
                    nc.vector.tensor_scalar_mul(stp[:], oh[:, 0:NS],
                                                stend[:, 0:1])
                    nc.vector.tensor_scalar_mul(enp[:], oh[:, N - NS:N],
                                                stend[:, 1:2])
                    sten = ps_f.tile([1, NS], F32, tag="f2")
                    nc.tensor.matmul(sten[:], ones45[:], stp[:],
                                     start=True, stop=False)
                    nc.tensor.matmul(sten[:], ones45[:], enp[:],
                                     start=False, stop=True)
                    nc.vector.tensor_copy(sten_s[:], sten[:])

                    sc1 = cp.tile([1, NS], F32, tag="sc1")
                    sc2 = cp.tile([1, NS], F32, tag="sc2")
                    lossa = cp.tile([1, NS], F32, tag="lossa")
                    lossb = cp.tile([1, NS], F32, tag="lossb")
                    nc.vector.tensor_add(sc1[:], em_sc[:], tr_sc[:])
                    nc.vector.tensor_add(sc2[:], sc1[:], sten_s[:])
                    nc.vector.tensor_tensor(out=lossa[:], in0=logZ[:],
                                            in1=sc2[:], op=ALU.subtract)
                    nc.scalar.activation(lossb[:], lossa[:], AF.Copy,
                                         bias=(S - 1) * LN45)
                    nc.sync.dma_start(out=d_loss[:], in_=lossb[:])

    nc.finalize()
    return nc


def _pack_wT(w, kchunks):
    # w: [M_out rows (gate units, reordered), K] ->
    # [128, (nm*kchunks)*128] tiles: tile (m*kchunks+ec) = w[mU, ecK].T
    M, K = w.shape
    nm = M // 128
    assert K == 128 * kchunks
    tiles = []
    for m in range(nm):
        for ec in range(kchunks):
            blk = w[m * 128:(m + 1) * 128, ec * 128:(ec + 1) * 128]
            tiles.append(np.ascontiguousarray(blk.T))
    return np.concatenate(tiles, axis=1)


def _perm_gates_ifog(w, hside):
    # torch gate order i,f,g,o (blocks of H) -> our chunk order i,f,o,g.
    # All-tanh trick scaling: gate pre-activations must arrive at the Act
    # as G = 8*a_ifo/2 resp. 8*a_g (Act scale=1/8 undoes the x8 fp8
    # encoding headroom): W_ifo x4, W_g x8; the h-side additionally
    # carries x0.5 because the stored state is h~ = 2h.
    i, f, g, o = np.split(w, 4, axis=0)
    s = 0.5 if hside else 1.0
    return np.concatenate([4.0 * s * i, 4.0 * s * f, 4.0 * s * o,
                           8.0 * s * g], axis=0)


def prepare_in_maps(**inputs):
    x = np.asarray(inputs["x"]).astype(np.int32)          # [32, 256]
    tags = np.asarray(inputs["tags"]).astype(np.int32)
    emb = np.asarray(inputs["emb"], dtype=np.float32)
    lin_w = np.asarray(inputs["lin_w"], dtype=np.float32)
    lin_b = np.asarray(inputs["lin_b"], dtype=np.float32)
    start_t = np.asarray(inputs["start_t"], dtype=np.float32)
    end_t = np.asarray(inputs["end_t"], dtype=np.float32)
    trans = np.asarray(inputs["trans"], dtype=np.float32)

    wihp = {0: _perm_gates_ifog(np.asarray(inputs["w_ih_f"], np.float32), False),
            1: _perm_gates_ifog(np.asarray(inputs["w_ih_b"], np.float32), False)}
    whhp = {0: _perm_gates_ifog(np.asarray(inputs["w_hh_f"], np.float32), True),
            1: _perm_gates_ifog(np.asarray(inputs["w_hh_b"], np.float32), True)}
    bp = {0: _perm_gates_ifog(np.asarray(inputs["b_f"], np.float32).reshape(-1, 1), False),
          1: _perm_gates_ifog(np.asarray(inputs["b_b"], np.float32).reshape(-1, 1), False)}

    wih_t = {dd: _pack_wT(wihp[dd], 2).astype(ml_dtypes.float8_e4m3) for dd in (0, 1)}
    whh_t = {dd: _pack_wT(whhp[dd], 4).astype(ml_dtypes.float8_e4m3) for dd in (0, 1)}

    # biasbc [128, 128]: col = d*64 + m*4 + s -> b_d[m*128 + p]
    biasbc = np.zeros((128, 128), np.float32)
    for dd in (0, 1):
        for m in range(16):
            col = bp[dd][m * 128:(m + 1) * 128, 0]
            for s in range(4):
                biasbc[:, dd * 64 + 4 * m + s] = col
    biasbc = biasbc.astype(ml_dtypes.bfloat16)

    # linT [128, 8*64]: chunk kc at col 64*kc = lin_w[:, kc*128:(kc+1)*128].T,
    # zero-padded to 64 cols so the PE's rounded 64-wide stationary tile
    # loads zeros (not adjacent SBUF) beyond column 45
    linT = np.zeros((128, 8 * 64), np.float32)
    for kc in range(8):
        linT[:, 64 * kc: 64 * kc + T] = 0.5 * lin_w[:, kc * 128:(kc + 1) * 128].T
    linT = linT.astype(ml_dtypes.bfloat16)

    id128 = np.eye(128, dtype=np.float32)

    # stacked (2 seqs per tile, second at partition 64) CRF operands:
    # epstk = block-diag(Ep, Ep), epstkT = block-diag(Ep^T, Ep^T) with
    # Ep = exp(trans - ln45); i45stk = block-diag(I45, I45);
    # stendstk = [start_t | end_t] stacked.
    Ep = np.exp(np.float64(trans) - np.log(45.0)).astype(np.float32)
    epstk = np.zeros((128, 128), np.float32)
    epstkT = np.zeros((128, 128), np.float32)
    i45stk = np.zeros((128, 45), np.float32)
    for r0 in (0, 64):
        epstk[r0:r0 + 45, r0:r0 + 45] = Ep
        epstkT[r0:r0 + 45, r0:r0 + 45] = Ep.T
        i45stk[r0:r0 + 45, :] = np.eye(45)
    epstk = epstk.astype(ml_dtypes.bfloat16)
    epstkT = epstkT.astype(ml_dtypes.bfloat16)
    i45stk = i45stk.astype(ml_dtypes.bfloat16)
    stendstk = np.zeros((128, 2), np.float32)
    linbstk = np.zeros((128, 1), np.float32)
    smask = np.zeros((128, 2), np.float32)
    smask[0:45, 0] = 1.0
    smask[64:109, 1] = 1.0
    for r0 in (0, 64):
        stendstk[r0:r0 + 45, 0] = start_t
        stendstk[r0:r0 + 45, 1] = end_t
        linbstk[r0:r0 + 45, 0] = lin_b

    in_maps = []
    for core in range(8):
        seqs = slice(4 * core, 4 * core + 4)
        xs = x[seqs]                                      # [4, 256]
        # xidx [128, 8]: col b, row r -> x[s=(r%4), t=(128b+r)//4]
        nflat = xs.T.reshape(-1)                          # n = 4t+s
        xidx = np.ascontiguousarray(nflat.reshape(8, 128).T).astype(np.int32)

        tg = tags[seqs]                                   # [4, 256]
        oh = np.zeros((T, N), np.float32)
        oh[tg.T.reshape(-1), np.arange(N)] = 1.0
        oh2 = np.zeros((T, N), np.float32)
        oh2[:, 0:N - NS] = oh[:, NS:N]

        in_maps.append({
            "emb": emb.astype(ml_dtypes.bfloat16),
            "xidx": xidx,
            "wihf": wih_t[0], "wihb": wih_t[1],
            "whhf": whh_t[0], "whhb": whh_t[1],
            "biasbc": biasbc,
            "linT": linT,
            "linb": lin_b.reshape(T, 1),
            "id128": id128,
            "idbf": np.eye(128, dtype=ml_dtypes.bfloat16),
            "trans": trans,
            "stend": np.stack([start_t, end_t], axis=1),
            "epstk": epstk,
            "epstkT": epstkT,
            "i45stk": i45stk,
            "stendstk": stendstk,
            "linbstk": linbstk,
            "smask": smask,
            "oh": oh,
            "oh2": oh2,
        })
    return in_maps


def get_nc():
    if "nc" not in _cached:
        _cached["nc"] = _build()
    return _cached["nc"]


def kernel(**inputs):
    in_maps = prepare_in_maps(**inputs)
    res = run_bass_kernel_spmd(get_nc(), in_maps, core_ids=list(range(8)))
    total = np.float64(0.0)
    for core in range(8):
        total += np.float64(res.results[core]["loss"]).sum()
    return np.float32(total / 32.0)
